# revision 64
# baseline (speedup 1.0000x reference)
"""Trainium2 Bass kernel for nn_MultiHeadAttention_30374008717799.

Reference computation (per problem): q = k = v = x @ Wq.T reshaped to 16 heads
of dim 64; causal softmax attention with scale 1/sqrt(1024); output re-merged
to [B, S, 1024].

Sharding: 8 cores = 4 batches x 2 head-groups (8 heads each). Each core gets
x[b] ([2048, 1024]) and its 512 rows of Wq, and produces out[b, :, 512g:512g+512].
No collectives needed; host reassembles.

Per-core algorithm (bf16 matmul operands, fp32 PSUM accumulation, fp32
softmax arithmetic on the scalar engine):
 - Transpose x and Wq on-chip via PE (contraction must sit on partitions).
 - qT[d, s] = WqT.T @ xT   (d-major q, feeds both score operands)
 - q_SD[s, d] (+ appended ones column) via PE transposes of qT, feeds AV lhsT.
 - Since k == q, the unnormalized exp'd score matrix U = exp(s/32) is
   symmetric, so tiles of U^T (what the AV matmul needs as its moving operand)
   are computed directly as scores tiles in [k, q] orientation -- no
   per-tile transposes of probabilities.
 - Softmax denominators come for free: the AV stationary operand is
   [q_SD | ones] ([128, 65]), so PSUM row 64 accumulates Z_q.
 - ctxT tiles [65, 512] are PE-transposed back to [s, d] orientation and
   scaled by 1/Z (per-partition scalar) into the output tile.
 - Engine balance: exp on ACT (the wall), matmuls/transposes on PE,
   PSUM->SBUF copies + masks + epilogue on DVE (GPSIMD measured ~2.7us per
   small elementwise op on HW -- never used), stores/loads on SP.
 - Software-pipelined schedule: prep (DMA, transpose, projection) for block
   sb+1/sb+2 is queued as micro-tasks and pumped between attention pairs so
   engine queues always hold ready work; epilogues are deferred into the
   next unit's micro stream to keep them off the PE critical path.
"""

import numpy as np

import concourse.bass as bass
import concourse.mybir as mybir
import concourse.tile as tile
from concourse.tile import ScopedClock
from concourse.bass_utils import run_bass_kernel_spmd

F32 = mybir.dt.float32
BF16 = mybir.dt.bfloat16
F8 = mybir.dt.float8e4
MM = BF16  # matmul operand dtype (AV path; scores run fp8 DoubleRow)
AF = mybir.ActivationFunctionType
DR = mybir.MatmulPerfMode.DoubleRow

S = 2048          # sequence length
E = 1024          # embed dim
DG = 512          # per-core output dims (8 heads x 64)
D = 64            # head dim
P = 128           # partitions
SC = S // P       # 16 s-chunks
EC = E // P       # 8 e-chunks
DC = DG // P      # 4 d-chunks (head pairs)
QB = S // 512     # 4 q-blocks of 512
SCALE = 1.0 / np.sqrt(1024.0)
USE_DR = False  # fp8 DoubleRow score matmuls (else bf16 from qT)


class TC(tile.TileContext):
    """TileContext adapted to this walrus build, which caps sync-waits at ONE
    per instruction: extra waits are peeled onto same-engine NoOps emitted
    just before the overloaded instruction, and the final drain gets the same
    treatment."""

    MAX_WAITS = 1

    def _lower_ordered_insts(self, ordered):
        for bb_name, insts in ordered.items():
            new_list = []
            for inst in insts:
                si = inst.sync_info
                if si is not None and si.on_wait and len(si.on_wait) > 1:
                    waits = list(si.on_wait)
                    upds = list(si.on_update) if si.on_update else []
                    inst.sync_info = mybir.SyncInfo(
                        on_wait=waits[-1:], on_update=upds
                    )
                    for w in waits[:-1]:
                        nop = mybir.InstNoOp(
                            name=f"I-wsplit-{self.nc.next_id()}", ins=[], outs=[]
                        )
                        nop.engine = inst.engine
                        nop.sync_info = mybir.SyncInfo(on_wait=[w], on_update=[])
                        new_list.append(nop)
                new_list.append(inst)
            insts[:] = new_list
        return super()._lower_ordered_insts(ordered)

    def _drain_and_barrier(self, tick_clock, wait_clock):
        nc = self.nc
        drain_inst = nc.sync.drain()
        wait_clock.add_sem_waits(
            drain_inst.ins, ScopedClock({None: tick_clock.global_clock})
        )
        si = drain_inst.ins.sync_info
        waits = list(si.on_wait) if si is not None and si.on_wait else []
        upds = list(si.on_update) if si is not None and si.on_update else []
        if len(waits) > self.MAX_WAITS:
            drain_inst.ins.sync_info = mybir.SyncInfo(
                on_wait=waits[: self.MAX_WAITS], on_update=upds
            )
            rest = waits[self.MAX_WAITS:]
            for k in range(0, len(rest), self.MAX_WAITS):
                extra = nc.sync.drain()
                extra.ins.sync_info = mybir.SyncInfo(
                    on_wait=rest[k : k + self.MAX_WAITS], on_update=[]
                )
        nc.all_engine_barrier()
        popped = nc._tile_sem_poison_stack.pop()
        assert popped is self._sem_poison
        nc.clear_and_free_semaphores(list(self.sems.allocated().values()))
        nc.all_engine_barrier()


def build(reps=1):
    nc = bass.Bass("TRN2", target_bir_lowering=False, debug=False)
    x_d = nc.declare_dram_parameter("x", [S, E], F32, isOutput=False)
    wq_d = nc.declare_dram_parameter("wq", [DG, E], F32, isOutput=False)
    tri_d = nc.declare_dram_parameter("tri", [P, P], F32, isOutput=False)
    iden_d = nc.declare_dram_parameter("iden", [P, P], F32, isOutput=False)
    out_d = nc.declare_dram_parameter("out", [S, DG], F32, isOutput=True)

    from contextlib import ExitStack

    with TC(nc) as tc, ExitStack() as es:
        cpool = es.enter_context(tc.tile_pool(name="consts", bufs=1))
        big = es.enter_context(tc.tile_pool(name="big", bufs=1))
        ut_pool = es.enter_context(tc.tile_pool(name="ut", bufs=8))
        ep_pool = es.enter_context(tc.tile_pool(name="ep", bufs=4))
        rc_pool = es.enter_context(tc.tile_pool(name="rc", bufs=4))
        wpool = es.enter_context(tc.tile_pool(name="wt", bufs=1))
        xs_pool = es.enter_context(tc.tile_pool(name="xs", bufs=8))
        xt_pool = es.enter_context(tc.tile_pool(name="xt", bufs=2))
        q8s_pool = es.enter_context(tc.tile_pool(name="q8s", bufs=2))
        psA = es.enter_context(tc.tile_pool(name="psA", bufs=2, space="PSUM"))
        psS = es.enter_context(tc.tile_pool(name="psS", bufs=2, space="PSUM"))
        psC = es.enter_context(tc.tile_pool(name="psC", bufs=2, space="PSUM"))

        # constants
        idf = cpool.tile([P, P], F32, name="idf")
        nc.sync.dma_start(idf[:], iden_d[:])
        idb = cpool.tile([P, P], MM, name="idb")
        nc.vector.tensor_copy(idb[:], idf[:])
        trf = cpool.tile([P, P], F32, name="trf")
        nc.sync.dma_start(trf[:], tri_d[:])
        trb = cpool.tile([P, P], MM, name="trb")
        nc.vector.tensor_copy(trb[:], trf[:])

        import contextlib
        loop_cm = tc.For_i(0, reps, 1) if reps > 1 else contextlib.nullcontext()
        es.enter_context(loop_cm)

        qT = big.tile([P, DC * S], MM, name="qT")
        # q_SD with ones column: layout [P, DC, SC, 130]:
        #   per (pair dc, k-chunk j): cols 0:64 head0 qsd, 64 ones,
        #                             65:129 head1 qsd, 129 ones
        qsd = big.tile([P, DC, SC, 130], MM, name="qsd")
        ctx_out = big.tile([P, SC * DG], F32, name="ctx_out")
        # fp8 q for DoubleRow score matmuls: head (dc, h2) lives in tile dc at
        # partitions h2*64 + p (base partition 0/64 only -- ISA constraint),
        # laid out [p, i, s] with head-dim d = p + 32*i. Both score operands
        # slice [32, 2, *] from here.
        q8t = (
            [big.tile([P, 2, S], F8, name=f"q8_{t}") for t in range(DC)]
            if USE_DR
            else [None] * DC
        )

        # ones columns of qsd: written once, read by every AV lhsT slice
        nc.vector.memset(qsd[:, :, :, 64:65], 1.0)
        nc.vector.memset(qsd[:, :, :, 129:130], 1.0)

        wqT = wpool.tile([P, EC * DG], MM, name="wqT")
        xs_tiles = {}
        xtb_tiles = {}
        from collections import deque
        pending = deque()  # prep micro-tasks, pumped between attention pairs

        def pump(n):
            for _ in range(n):
                if not pending:
                    return
                pending.popleft()()

        def flush():
            pump(len(pending))

        # ---------- emit helpers (software-pipelined schedule) ----------
        ws_tiles = {}

        def emit_wq_dma(dc):
            ws = xs_pool.tile([P, E], F32, name="ws", tag="xs")
            nc.sync.dma_start(ws[:], wq_d[dc * P : (dc + 1) * P, :])
            ws_tiles[dc] = ws

        def queue_wqT(dc):
            def piece(ec0):
                ws = ws_tiles[dc]
                pt4 = psA.tile([P, 512], F32, name="ptw", tag="pt")
                for c in range(4):
                    ec = ec0 + c
                    nc.tensor.transpose(
                        pt4[:, c * P : (c + 1) * P], ws[:, ec * P : (ec + 1) * P], idf[:]
                    )
                dst = wqT[:].rearrange("p (e d) -> p e d", d=DG)[
                    :, ec0 : ec0 + 4, dc * P : (dc + 1) * P
                ]
                nc.vector.tensor_copy(dst, pt4[:].rearrange("p (e c) -> p e c", c=P))
            for ec0 in range(0, EC, 4):
                pending.append(lambda ec0=ec0: piece(ec0))

        def emit_dma_x(sb, k):
            sc = 4 * sb + k
            xs = xs_pool.tile([P, E], F32, name="xs", tag="xs")
            nc.sync.dma_start(xs[:], x_d[sc * P : (sc + 1) * P, :])
            xs_tiles[(sb, k)] = xs

        def queue_xpose(sb, k):
            def grab():
                if k == 0:
                    xtb_tiles[sb] = xt_pool.tile([P, EC * 512], MM, name="xtb")
            pt_box = {}
            def xpose4(ec0):
                xs = xs_tiles[(sb, k)]
                pt4 = psA.tile([P, 512], F32, name="ptx", tag="pt")
                pt_box[ec0] = pt4
                for c in range(4):
                    ec = ec0 + c
                    nc.tensor.transpose(
                        pt4[:, c * P : (c + 1) * P], xs[:, ec * P : (ec + 1) * P], idf[:]
                    )
            def copy4(ec0):
                xtb = xtb_tiles[sb]
                dst = xtb[:].rearrange("p (e c) -> p e c", c=512)[
                    :, ec0 : ec0 + 4, k * P : (k + 1) * P
                ]
                nc.vector.tensor_copy(
                    dst, pt_box.pop(ec0)[:].rearrange("p (e c) -> p e c", c=P)
                )
            pending.append(grab)
            for ec0 in range(0, EC, 4):
                pending.append(lambda ec0=ec0: xpose4(ec0))
                pending.append(lambda ec0=ec0: copy4(ec0))

        def queue_proj(sb, dc):
            pq_box = {}
            def mm(ec0):
                if ec0 == 0:
                    pq_box["pq"] = psA.tile([P, 512], F32, name="pq", tag="pt")
                pq = pq_box["pq"]
                for ec in (ec0, ec0 + 1):
                    nc.tensor.matmul(
                        pq[:],
                        lhsT=wqT[:, ec * DG + dc * P : ec * DG + (dc + 1) * P],
                        rhs=xtb_tiles[sb][:, ec * 512 : (ec + 1) * 512],
                        start=(ec == 0),
                        stop=(ec == EC - 1),
                    )
            def qcopy():
                nc.vector.tensor_copy(
                    qT[:, dc * S + sb * 512 : dc * S + (sb + 1) * 512], pq_box["pq"]
                )
            def qsdT():
                pt4 = psA.tile([P, 512], MM, name="ptq", tag="pt")
                for j4 in range(4):
                    j = 4 * sb + j4
                    nc.tensor.transpose(
                        pt4[:, j4 * P : (j4 + 1) * P],
                        qT[:, dc * S + j * P : dc * S + (j + 1) * P],
                        idb[:],
                    )
                pq_box["pt4"] = pt4
            def qsdCopy():
                dst = qsd[:, dc, 4 * sb : 4 * sb + 4, :].rearrange(
                    "p j (g c) -> p j g c", g=2
                )[:, :, :, 0:64]
                srcp = pq_box["pt4"][:].rearrange("p (j g c) -> p j g c", j=4, g=2)
                nc.vector.tensor_copy(dst, srcp)
            def q8stage():
                q8s = q8s_pool.tile([P, 512], F8, name="q8s")
                nc.vector.tensor_copy(q8s[:], pq_box["pq"])
                pq_box["q8s"] = q8s
            def q8move():
                # partition regroup via SBUF->SBUF DMA: rows 64*h2+32*i+p of
                # the stage tile land at rows 64*h2+p, free plane i
                q8s = pq_box["q8s"]
                for h2 in range(2):
                    for i2 in range(2):
                        r0 = 64 * h2 + 32 * i2
                        nc.sync.dma_start(
                            q8t[dc][
                                64 * h2 : 64 * h2 + 32,
                                i2,
                                sb * 512 : (sb + 1) * 512,
                            ],
                            q8s[r0 : r0 + 32, :],
                        )
            for ec0 in range(0, EC, 2):
                pending.append(lambda ec0=ec0: mm(ec0))
            if USE_DR:
                pending.append(q8stage)
                pending.append(q8move)
            pending.append(qcopy)
            pending.append(qsdT)
            pending.append(qsdCopy)

        def emit_attn(sb, dc):
            # attention for q-block i = sb, head-pair dc (both heads h2).
            # The two heads of a pair are interleaved inside the j-loop: their
            # K=64 score matmuls sit in disjoint PE row-groups (base partition
            # 0 vs 64) and issue back-to-back, so they run concurrently; the
            # per-head exp/AV chains ping-pong PE against ACT.
            i = sb
            njj = 4 * i + 4
            cps = [
                psC.tile([P, 512], F32, name=f"cps{h2}", tag="cps") for h2 in range(2)
            ]
            for j0 in range(0, njj, 2):
                # column layout: region u=0 at [ce0:512], u=1 packed at
                # [512 : 1024-ce1], so the exp below is a single contiguous call
                ces = [max(0, (j0 + u) * P - i * 512) for u in range(2)]
                cbs = [0, 512 - ces[1]]
                sts = []
                for h2 in range(2):
                    q8 = q8t[dc][64 * h2 : 64 * h2 + 32, :, :] if USE_DR else None
                    pb = h2 * 64
                    st = psS.tile([P, 1024], F32, name="st", tag="st")
                    sts.append(st)
                    for u in range(2):
                        jj = j0 + u
                        ce = ces[u]
                        if USE_DR:
                            nc.tensor.matmul(
                                st[:, cbs[u] + ce : cbs[u] + 512],
                                lhsT=q8[:, :, jj * P : (jj + 1) * P],
                                rhs=q8[:, :, i * 512 + ce : (i + 1) * 512],
                                start=True,
                                stop=True,
                                perf_mode=DR,
                            )
                        else:
                            nc.tensor.matmul(
                                st[:, cbs[u] + ce : cbs[u] + 512],
                                lhsT=qT[pb : pb + 64, dc * S + jj * P : dc * S + (jj + 1) * P],
                                rhs=qT[pb : pb + 64, dc * S + i * 512 + ce : dc * S + (i + 1) * 512],
                                start=True,
                                stop=True,
                            )
                uts = []
                for h2 in range(2):
                    ut = ut_pool.tile([P, 1024], MM, name="ut")
                    uts.append(ut)
                    nc.scalar.activation(
                        ut[:, ces[0] : 1024 - ces[1]],
                        sts[h2][:, ces[0] : 1024 - ces[1]],
                        AF.Exp,
                        scale=SCALE,
                    )
                for h2 in range(2):
                    ut = uts[h2]
                    mreg = [
                        cbs[u] + ces[u]
                        for u in range(2)
                        if j0 + u >= 4 * i  # diagonal block: triangle mask
                    ]
                    if (
                        len(mreg) == 2
                        and mreg[1] - mreg[0] in (256, 512)
                        and mreg[0] + 2 * (mreg[1] - mreg[0]) <= 1024
                    ):
                        # both diag masks in one strided DVE op
                        stride = mreg[1] - mreg[0]
                        mv = ut[:, mreg[0] : mreg[0] + 2 * stride].rearrange(
                            "p (g c) -> p g c", c=stride
                        )[:, :, 0:P]
                        trv = trb[:].rearrange("p (g c) -> p g c", g=1).broadcast_to(
                            [P, 2, P]
                        )
                        nc.vector.tensor_mul(mv, mv, trv)
                    else:
                        for m0 in mreg:
                            nc.vector.tensor_mul(
                                ut[:, m0 : m0 + P], ut[:, m0 : m0 + P], trb[:]
                            )
                for h2 in range(2):
                    for u in range(2):
                        jj = j0 + u
                        ce = ces[u]
                        nc.tensor.matmul(
                            cps[h2][0:65, ce:512],
                            lhsT=qsd[:, dc, jj, h2 * 65 : h2 * 65 + 65],
                            rhs=uts[h2][:, cbs[u] + ce : cbs[u] + 512],
                            start=(jj == 0),
                            stop=(jj == njj - 1),
                        )
                pump(4)  # interleave prep micro-tasks between attention pairs
            # epilogue part 1 (inline): drain cps to SBUF on DVE. Part 2 (the
            # PE transposes + normalize) is queued by the caller into the next
            # unit's micro stream so it never blocks this unit's PE queue.
            csbs = []
            for h2 in range(2):
                csb = ep_pool.tile([65, 512], F32, name="csb")
                nc.vector.tensor_copy(csb[:], cps[h2][0:65, :])
                csbs.append(csb)
            return csbs

        def queue_epilogue(sb, dc, csbs):
            i = sb
            def epi(h2):
                csb = csbs[h2]
                ptc = psA.tile([P, 4 * 65], F32, name="ptc", tag="pt")
                ptcv = ptc[:].rearrange("p (c k) -> p c k", k=65)
                for c in range(4):
                    nc.tensor.transpose(
                        ptcv[:, c, :], csb[:, c * P : (c + 1) * P], idf[0:65, 0:65]
                    )
                rc = rc_pool.tile([P, 4], F32, name="rc")
                rcv = rc[:].rearrange("p (c k) -> p c k", k=1)
                nc.vector.reciprocal(rcv, ptcv[:, :, 64:65])
                h = 2 * dc + h2
                dst = ctx_out[:].rearrange("p (s g) -> p s g", g=DG)[
                    :, 4 * i : 4 * i + 4, h * D : (h + 1) * D
                ]
                nc.vector.tensor_mul(dst, ptcv[:, :, 0:64], rcv.broadcast_to([P, 4, D]))
            for h2 in range(2):
                pending.append(lambda h2=h2: epi(h2))

        def emit_stores(sb):
            for c in range(4):
                sc = 4 * sb + c
                nc.sync.dma_start(
                    out_d[sc * P : (sc + 1) * P, :],
                    ctx_out[:, sc * DG : (sc + 1) * DG],
                )

        # ---------- prologue: x block 0 + wq DMAs, block-0 transposes,
        # proj(0,0) inline; proj(0,1..3) + wq transposes pump inside attn(0) ----------
        emit_dma_x(0, 0)
        emit_wq_dma(0)
        queue_wqT(0)
        for k in range(1, 4):
            emit_dma_x(0, k)
        for dc in range(1, 4):
            emit_wq_dma(dc)
        for k in range(4):
            queue_xpose(0, k)
        queue_proj(0, 0)
        flush()
        for dc in range(1, 4):
            queue_wqT(dc)
            queue_proj(0, dc)
        for k in range(4):
            emit_dma_x(1, k)
            queue_xpose(1, k)

        # ---------- main loop: 2-deep prep pipeline. During attention(sb):
        # the previous unit's epilogue, proj/q8/qsd of block sb+1, and
        # DMA+transpose of block sb+2 are pumped between attention pairs ----------
        prev_epi = None
        for sb in range(4):
            for dc in range(DC):
                if prev_epi is not None:
                    queue_epilogue(*prev_epi)
                if sb < 3:
                    queue_proj(sb + 1, dc)
                if sb < 2:
                    if dc == 0:
                        emit_dma_x(sb + 2, 0)
                        emit_dma_x(sb + 2, 1)
                    elif dc == 1:
                        emit_dma_x(sb + 2, 2)
                        queue_xpose(sb + 2, 0)
                    elif dc == 2:
                        emit_dma_x(sb + 2, 3)
                        queue_xpose(sb + 2, 1)
                    else:
                        queue_xpose(sb + 2, 2)
                        queue_xpose(sb + 2, 3)
                csbs = emit_attn(sb, dc)
                prev_epi = (sb, dc, csbs)
                flush()
                if dc == 0 and sb > 0:
                    emit_stores(sb - 1)
        queue_epilogue(*prev_epi)
        flush()
        emit_stores(3)

    return nc


def _host_consts():
    tri = np.triu(np.ones((P, P), dtype=np.float32))  # tri[k, q] = 1 iff k <= q
    iden = np.eye(P, dtype=np.float32)
    return tri, iden


def make_in_maps(x, Wq):
    tri, iden = _host_consts()
    in_maps = []
    for c in range(8):
        b, g = c // 2, c % 2
        in_maps.append(
            {
                "x": np.ascontiguousarray(np.asarray(x[b], dtype=np.float32)),
                "wq": np.ascontiguousarray(
                    np.asarray(Wq[g * DG : (g + 1) * DG], dtype=np.float32)
                ),
                "tri": tri,
                "iden": iden,
            }
        )
    return in_maps


_NC_CACHE = {}


def _get_nc():
    if "nc" not in _NC_CACHE:
        _NC_CACHE["nc"] = build()
    return _NC_CACHE["nc"]


def run(x, Wq, **spmd_kwargs):
    x = np.asarray(x, dtype=np.float32)
    Wq = np.asarray(Wq, dtype=np.float32)
    nc = _get_nc()
    in_maps = make_in_maps(x, Wq)
    kr = run_bass_kernel_spmd(nc, in_maps, list(range(8)), **spmd_kwargs)
    out = np.empty((4, S, E), dtype=np.float32)
    for c in range(8):
        b, g = c // 2, c % 2
        out[b, :, g * DG : (g + 1) * DG] = kr.results[c]["out"]
    return out.reshape(4, S, E), kr


def kernel(x, Wq):
    out, _ = run(x, Wq)
    return out
